# revision 37
# baseline (speedup 1.0000x reference)
"""Trainium2 Bass kernel for nn_AnswerDecoder (LSTM decoder + vocab projection).

Sharding: vocab-parallel across 8 NeuronCores (each core owns V/8 = 2500 rows
of W_vocab and produces logits[:, :, k*2500:(k+1)*2500]); the LSTM itself is
replicated on every core (its PE cost is set by weight-streaming, not batch
size, so replication is free). No collectives.

Numerics: all matmuls bf16 with fp32 PSUM accumulation; LSTM cell state c is
kept fp32; h is rounded to bf16 each step (validated: end-to-end rel err
~2.5e-3 vs fp32 reference). Logits leave the device as bf16 and are cast to
fp32 on the host.

v5 structure (vs the 398us v3):
 - The x-projection (x @ W_ih^T + b_ih + b_hh, gate-permuted) is computed on
   the host in fp32 and streamed per 128-token chunk on the GpSimd (SWDGE)
   DMA queue, 6 chunks deep. The per-step PSUM inject stays as col-tiled
   identity-selector pairs (a full-array M=128 inject was tried and broke
   the col-group concurrency of the following gate pairs: +30us).
 - Vocab filler is balanced to exactly 10 matmuls per step (kc-accumulation
   of a tile may span the two steps of its chunk), so the PE no longer runs
   dry before the h transposes on the lighter steps.
 - tanh(c) is computed in two 128-col halves so h's first half (and its PE
   transpose) starts ~400ns earlier.
 - Startup: whh/wvt are split across the sync and SWDGE queues so the first
   gate matmul starts ~10us earlier.
"""
import os
import sys
import types

import numpy as np

import concourse.bass as bass
import concourse.bacc as bacc
import concourse.mybir as mybir
from concourse import tile
from concourse.bass_utils import run_bass_kernel_spmd

dt = mybir.dt
AF = mybir.ActivationFunctionType

B, T = 64, 64
Q, E, H, V = 512, 256, 512, 20000
NCORES = 8
VS = V // NCORES          # 2500 vocab rows per core
TB = T * B                # 4096 tokens
NVT = 5                   # vocab N-tiles per 128-token chunk
VT = VS // NVT            # 500
NCH = TB // 128           # 32 token chunks
XP_PREFETCH = 6           # xp chunk DMA lookahead (== xps pool depth)
START_IDX = 1


def _gate_perm():
    """new gate-column index -> original gate-column index.

    bank0 = [i|g] (both inputs of the early i*g product), bank1 = [f|o]
    (consumed late in the chain), so the c-update critical path starts as
    soon as bank1's matmuls land."""
    gate_of = {0: (0, 2), 1: (1, 3)}   # bank -> (q for j<256, q for j>=256)
    perm = np.empty(4 * H, dtype=np.int64)
    for bank in range(2):
        for hh in range(2):
            for j in range(512):
                q = gate_of[bank][1 if j >= 256 else 0]
                u = 256 * hh + (j % 256)
                perm[bank * 1024 + hh * 512 + j] = q * H + u
    return perm


def build(nc):
    f32, bf16 = dt.float32, dt.bfloat16

    xpt_d = nc.declare_dram_parameter("xpt", [TB, 4 * H], bf16, isOutput=False)
    whh_d = nc.declare_dram_parameter("whh", [H, 4 * H], bf16, isOutput=False)
    h0t_d = nc.declare_dram_parameter("h0t", [128, 4 * B], bf16, isOutput=False)
    c0_d = nc.declare_dram_parameter("c0t", [128, 256], bf16, isOutput=False)
    ident_d = nc.declare_dram_parameter("ident", [128, 128], bf16, isOutput=False)
    wvt_d = nc.declare_dram_parameter("wvt", [H + 1, VS], bf16, isOutput=False)
    out_d = nc.declare_dram_parameter("out", [TB, VS], bf16, isOutput=True)

    with tile.TileContext(nc) as tc:
        with (
            tc.tile_pool(name="const", bufs=1) as const,
            tc.tile_pool(name="work", bufs=2) as work,
            tc.tile_pool(name="hbf", bufs=2) as hpool,
            tc.tile_pool(name="xps", bufs=XP_PREFETCH) as xps,
            tc.tile_pool(name="stage", bufs=2) as stpool,
            tc.tile_pool(name="pgate", bufs=2, space="PSUM") as pg,
            tc.tile_pool(name="ptrans", bufs=1, space="PSUM") as pt,
            tc.tile_pool(name="pvocab", bufs=3, space="PSUM") as pv,
        ):
            # ---- constant uploads, split across the two DMA queues so the
            # recurrence can start as soon as whh + xp chunk 0 land ---------
            whh = const.tile([128, 4 * 4 * H], bf16)        # [128, (kc, gatecol)]
            whh_v = whh[:].rearrange("p (c n) -> p c n", c=4)
            whh_dv = whh_d[:].rearrange("(c p) n -> p c n", p=128)
            wvt = const.tile([128, 4 * VS], bf16)           # [128, (kc, vocab)]
            wvt_v = wvt[:].rearrange("p (c n) -> p c n", c=4)
            wvt_dv = wvt_d[0:H, :].rearrange("(c p) n -> p c n", p=128)

            xp_tiles = {}

            def emit_xp_dma(c, eng=None):
                xp_tiles[c] = xps.tile([128, 4 * H], bf16, tag="xp", name=f"xp{c}")
                (eng or nc.gpsimd).dma_start(
                    xp_tiles[c][:], xpt_d[128 * c : 128 * c + 128, :]
                )

            ident = const.tile([128, 128], bf16)
            nc.sync.dma_start(ident[:], ident_d[:])
            h0T = const.tile([128, 4 * B], bf16)
            nc.sync.dma_start(h0T[:], h0t_d[:])
            c_t = const.tile([128, 256], bf16)
            nc.sync.dma_start(c_t[:], c0_d[:])
            emit_xp_dma(0)                                  # gpsimd queue
            nc.sync.dma_start(whh_v[:, 0:1, :], whh_dv[:, 0:1, :])
            nc.gpsimd.dma_start(whh_v[:, 2:3, :], whh_dv[:, 2:3, :])
            nc.sync.dma_start(whh_v[:, 1:2, :], whh_dv[:, 1:2, :])
            nc.gpsimd.dma_start(whh_v[:, 3:4, :], whh_dv[:, 3:4, :])
            emit_xp_dma(1, nc.sync)
            nc.gpsimd.dma_start(wvt_v[:, 2:3, :], wvt_dv[:, 2:3, :])
            nc.sync.dma_start(wvt_v[:, 0:1, :], wvt_dv[:, 0:1, :])
            nc.gpsimd.dma_start(wvt_v[:, 3:4, :], wvt_dv[:, 3:4, :])
            nc.sync.dma_start(wvt_v[:, 1:2, :], wvt_dv[:, 1:2, :])
            for c0_ in range(2, XP_PREFETCH):
                emit_xp_dma(c0_, nc.sync if c0_ % 2 else None)

            H_allT = const.tile([128, 4 * TB], bf16)        # [128, (kc, token)]

            # ---- PE warmup: ~5us of dummy matmuls so the HAM clock gate
            # un-throttles (1.2 -> 2.4 GHz) while the parameter DMAs stream,
            # instead of 3.4us into the real recurrence. Results unread.
            wu = pt.tile([128, 256], f32, tag="pst", name="wu")
            for w in range(40):
                r = (w % 2) * 128
                nc.tensor.matmul(
                    wu[:, r : r + 128],
                    lhsT=ident[:],
                    rhs=ident[:],
                    start=True,
                    stop=True,
                )

            # ---- vocab projection: a flat queue of (tile, kc) matmuls,
            # drained at exactly 10 per step --------------------------------
            vocab_psum = {}
            st_tiles = {}
            mm_queue = []            # pending (m, vl, kc) in emission order

            def queue_vocab_chunk(m):
                for vl in range(NVT):
                    for kc in range(4):
                        mm_queue.append((m, vl, kc))

            def emit_vocab_mms(n):
                for _ in range(min(n, len(mm_queue))):
                    m, vl, kc = mm_queue.pop(0)
                    if kc == 0:
                        vocab_psum[(m, vl)] = pv.tile(
                            [128, VT], f32, tag="psv", name=f"psv{m}_{vl}"
                        )
                    nc.tensor.matmul(
                        vocab_psum[(m, vl)][:],
                        lhsT=H_allT[:, kc * TB + 128 * m : kc * TB + 128 * m + 128],
                        rhs=wvt[:, kc * VS + vl * VT : kc * VS + vl * VT + VT],
                        start=(kc == 0),
                        stop=(kc == 3),
                    )

            def emit_vocab_stage(m, vls):
                # plain PSUM->SBUF cast (b_vocab is folded in on the host);
                # 2 tiles per chunk on DVE, 3 on ACT so neither saturates
                if m not in st_tiles:
                    st_tiles[m] = stpool.tile(
                        [128, VS], bf16, tag="st", name=f"st{m}"
                    )
                st = st_tiles[m]
                for vl in vls:
                    dst = st[:, vl * VT : (vl + 1) * VT]
                    src = vocab_psum.pop((m, vl))[:]
                    if vl in (0, 2, 4):
                        nc.vector.tensor_copy(dst, src)
                    else:
                        nc.scalar.copy(dst, src)

            def emit_vocab_out(m):
                nc.sync.dma_start(out_d[128 * m : 128 * m + 128, :], st_tiles.pop(m)[:])

            # ---- the 64 LSTM steps ---------------------------------------------
            # Each step's two PSUM banks are opened by the xp inject (selector
            # columns of the identity pick the step's 64 token rows out of the
            # 128-token xp chunk). The inject has no h dependency, so step
            # t+1's injects are emitted BEFORE step t's h transposes: they
            # fill the PE during the ~0.4us the transposes wait on h_bf.
            def emit_inject(t):
                s = t % 2
                xpc = xp_tiles[t // 2]
                p0 = pg.tile([128, 512], f32, tag="psg")
                p1 = pg.tile([128, 512], f32, tag="psg")
                for bank, psg in ((0, p0), (1, p1)):
                    for hh in range(2):
                        n0 = bank * 1024 + hh * 512
                        nc.tensor.matmul(
                            psg[64 * hh : 64 * hh + 64, :],
                            lhsT=ident[:, s * 64 : s * 64 + 64],
                            rhs=xpc[:, n0 : n0 + 512],
                            start=True,
                            stop=False,
                            tile_position=(0, 64 * hh),
                            skip_group_check=True,
                        )
                return p0, p1

            psg_next = emit_inject(0)
            for t in range(T):
                psg0, psg1 = psg_next
                cch, s = t // 2, t % 2
                if s == 0 and cch > 0:
                    xp_tiles.pop(cch - 1)

                def lhs_h(kc, t=t):
                    if t == 0:
                        return h0T[:, kc * 64 : (kc + 1) * 64]
                    c0 = kc * TB + 64 * (t - 1)
                    return H_allT[:, c0 : c0 + 64]

                # interleave the two column-tile chains (hh=0 on cols 0-63,
                # hh=1 on cols 64-127): adjacent matmuls hit different col
                # groups and run concurrently in the PE array. N=512 per MM
                # is the sweet spot: the pair's two M=64 weight loads
                # (2 x 53ns) exactly hide under the 213ns stream; smaller N
                # goes LDWEIGHTS-bound (measured: N=256 regions cost +120us).
                for bank, psg in ((0, psg0), (1, psg1)):
                    for kc in (0, 2, 1, 3):
                        for hh in range(2):
                            n0 = bank * 1024 + hh * 512
                            nc.tensor.matmul(
                                psg[64 * hh : 64 * hh + 64, :],
                                lhsT=lhs_h(kc),
                                rhs=whh[:, kc * 2048 + n0 : kc * 2048 + n0 + 512],
                                start=False,
                                stop=(kc == 3),
                                tile_position=(0, 64 * hh),
                                skip_group_check=True,
                            )

                # activations: bank0 = [i|g] (lands first), bank1 = [f|o].
                # The whole elementwise chain runs in bf16 (2x DVE modes;
                # validated 3.9e-3 end-to-end rel err vs 2.1e-3 with f32 c).
                s_ig = work.tile([128, 512], bf16, tag="s_ig")
                s_fo = work.tile([128, 512], bf16, tag="s_fo")
                igt = work.tile([128, 256], bf16, tag="igt")
                fct = work.tile([128, 256], bf16, tag="fct")
                tct = work.tile([128, 256], bf16, tag="tct")
                h_bf = hpool.tile([128, 256], bf16, tag="h")

                nc.scalar.activation(s_ig[:, 0:256], psg0[:, 0:256], AF.Sigmoid)
                nc.scalar.activation(s_ig[:, 256:512], psg0[:, 256:512], AF.Tanh)
                nc.vector.tensor_mul(igt[:], s_ig[:, 0:256], s_ig[:, 256:512])
                # sigmoid(f) in 128-col halves so fct0 starts one ACT-op
                # earlier; c-update halves pipeline with the tanh halves below
                nc.scalar.activation(s_fo[:, 0:128], psg1[:, 0:128], AF.Sigmoid)
                nc.scalar.activation(s_fo[:, 128:256], psg1[:, 128:256], AF.Sigmoid)
                nc.scalar.activation(s_fo[:, 256:512], psg1[:, 256:512], AF.Sigmoid)
                for ui in range(2):
                    cs = slice(ui * 128, (ui + 1) * 128)
                    nc.vector.tensor_mul(fct[:, cs], s_fo[:, cs], c_t[:, cs])
                    nc.vector.tensor_add(c_t[:, cs], fct[:, cs], igt[:, cs])

                # lagged vocab matmuls fill the PE while this step's
                # activation chain runs; exactly 10 MMs per step
                if t >= 2 and t % 2 == 0:
                    queue_vocab_chunk(t // 2 - 1)
                emit_vocab_mms(10)
                if t + 1 < T:
                    psg_next = emit_inject(t + 1)

                # tanh(c) in two halves; h's first half (and its transpose)
                # starts while the second tanh half is still on ACT
                pst = pt.tile([128, 256], f32, tag="pst")
                H_v = H_allT[:].rearrange("p (c n) -> p c n", c=4)
                for ui in range(2):
                    nc.scalar.activation(
                        tct[:, ui * 128 : (ui + 1) * 128],
                        c_t[:, ui * 128 : (ui + 1) * 128],
                        AF.Tanh,
                    )
                    nc.vector.tensor_mul(
                        h_bf[:, ui * 128 : (ui + 1) * 128],
                        s_fo[:, 256 + ui * 128 : 256 + (ui + 1) * 128],
                        tct[:, ui * 128 : (ui + 1) * 128],
                    )
                    # transpose as a column-tiled M=64 pair: ~3x cheaper than
                    # one M=128 matmul, and it keeps the PE in col-tiled
                    # config (a full-width->col-tiled switch costs ~120ns on
                    # the next gate/inject matmul)
                    for j in range(2):
                        nc.tensor.matmul(
                            pst[64 * j : 64 * j + 64, ui * 128 : (ui + 1) * 128],
                            lhsT=h_bf[:, ui * 128 + 64 * j : ui * 128 + 64 * j + 64],
                            rhs=ident[:],
                            start=True,
                            stop=True,
                            tile_position=(0, 64 * j),
                            skip_group_check=True,
                        )
                    if ui == 0:
                        nc.vector.tensor_copy(
                            H_v[:, 0:3:2, 64 * t : 64 * t + 64],
                            pst[:, 0:128].rearrange("p (c n) -> p c n", c=2),
                        )
                    else:
                        nc.scalar.copy(
                            H_v[:, 1:4:2, 64 * t : 64 * t + 64],
                            pst[:, 128:256].rearrange("p (c n) -> p c n", c=2),
                        )

                # next xp chunk DMA (SWDGE queue): chunk cch+PREFETCH reuses
                # chunk cch's buffer, so emit only after this chunk's last
                # inject (the s==1 one, earlier this iteration) is in the IR
                if s == 1 and cch + XP_PREFETCH < NCH:
                    emit_xp_dma(cch + XP_PREFETCH)
                # staging casts sit behind the chain ops in each engine FIFO
                if t >= 2:
                    m = t // 2 - 1
                    if t % 2 == 0:
                        emit_vocab_stage(m, (0, 1))
                    else:
                        emit_vocab_stage(m, (2, 3, 4))
                        emit_vocab_out(m)

            # tail: last vocab chunk, stage->DMA pipelined per tile
            m = NCH - 1
            queue_vocab_chunk(m)
            emit_vocab_mms(20)
            for vl in range(NVT):
                # scalar HWDGE queue: the sync queue is still draining chunk
                # 30's full-row out DMA at this point
                emit_vocab_stage(m, (vl,))
                nc.scalar.dma_start(
                    out_d[128 * m : 128 * m + 128, vl * VT : (vl + 1) * VT],
                    st_tiles[m][:, vl * VT : (vl + 1) * VT],
                )
            st_tiles.pop(m)


def _host_prep(inputs):
    import ml_dtypes

    bf = ml_dtypes.bfloat16
    f32 = np.float32

    qv = inputs["question_vectors"].astype(f32)
    emb = inputs["emb_table"].astype(f32)
    W_h, W_c = inputs["W_h"].astype(f32), inputs["W_c"].astype(f32)
    W_ih, W_hh = inputs["W_ih"].astype(f32), inputs["W_hh"].astype(f32)
    b_ih, b_hh = inputs["b_ih"].astype(f32), inputs["b_hh"].astype(f32)
    W_vocab, b_vocab = inputs["W_vocab"].astype(f32), inputs["b_vocab"].astype(f32)
    answers = inputs["answers"]

    perm = _gate_perm()
    whh = np.ascontiguousarray(W_hh.T[:, perm]).astype(bf)      # [512, 2048]

    # teacher-forced inputs gathered on host, then the x-projection
    # (x @ W_ih^T + b_ih + b_hh) in fp32, gate-permuted: [TB, 2048]
    xs = np.concatenate(
        [
            np.broadcast_to(emb[START_IDX], (1, B, E)),
            emb[answers[:, :-1]].transpose(1, 0, 2),
        ],
        axis=0,
    ).reshape(TB, E)
    xpt = np.ascontiguousarray(
        (xs @ W_ih.T + (b_ih + b_hh))[:, perm]
    ).astype(bf)                                                # [TB, 2048]

    # initial state projections, pre-tiled for the device layouts
    h0 = qv @ W_h.T                                             # [B, H]
    c0 = qv @ W_c.T                                             # [B, H]
    h0t = np.ascontiguousarray(
        h0.T.reshape(4, 128, B).transpose(1, 0, 2).reshape(128, 4 * B)
    ).astype(bf)                                                # [128,(kc,b)]
    c0t = np.ascontiguousarray(
        c0.reshape(B, 2, 256).transpose(1, 0, 2).reshape(128, 256)
    ).astype(bf)                                                # [(hh,b),256]

    ident = np.eye(128, dtype=bf)

    common = dict(xpt=xpt, whh=whh, h0t=h0t, c0t=c0t, ident=ident)
    in_maps = []
    for k in range(NCORES):
        wvt = np.concatenate(
            [W_vocab[k * VS : (k + 1) * VS].T, b_vocab[None, k * VS : (k + 1) * VS]],
            axis=0,
        ).astype(bf)                                        # [513, 2500]
        in_maps.append(dict(common, wvt=wvt))
    return in_maps


def _install_ntff_hook():
    """Shim antenv.axon_hooks (absent in this image) so BASS_TRACE=1 works."""
    if "antenv.axon_hooks" in sys.modules:
        return
    try:
        mod = types.ModuleType("antenv.axon_hooks")
        mod._hook = None
        mod.set_axon_ntff_profile_hook = lambda h: setattr(mod, "_hook", h)
        mod.get_axon_ntff_profile_hook = lambda: mod._hook
        sys.modules["antenv.axon_hooks"] = mod
        from trn_agent_boot.trn_boot import _ntff_profile_via_ctypes

        mod.set_axon_ntff_profile_hook(
            _ntff_profile_via_ctypes("/opt/axon/libaxon_pjrt.so")
        )
    except Exception:
        sys.modules.pop("antenv.axon_hooks", None)


def kernel(**inputs):
    inputs = {k: np.asarray(v) for k, v in inputs.items()}
    if os.environ.get("BASS_TRACE"):
        _install_ntff_hook()

    in_maps = _host_prep(inputs)

    nc = bacc.Bacc("TRN2", target_bir_lowering=False, debug=False, num_devices=NCORES)
    build(nc)
    nc.compile()

    res = run_bass_kernel_spmd(nc, in_maps, core_ids=list(range(NCORES)))
    kernel._last_result = res

    b_vocab = inputs["b_vocab"].astype(np.float32)
    outs = [
        res.results[k]["out"].astype(np.float32).reshape(T, B, VS).transpose(1, 0, 2)
        + b_vocab[k * VS : (k + 1) * VS]
        for k in range(NCORES)
    ]
    return np.concatenate(outs, axis=2)


# revision 39
# speedup vs baseline: 1.0067x; 1.0067x over previous
"""Trainium2 Bass kernel for nn_AnswerDecoder (LSTM decoder + vocab projection).

Sharding: vocab-parallel across 8 NeuronCores (each core owns V/8 = 2500 rows
of W_vocab and produces logits[:, :, k*2500:(k+1)*2500]); the LSTM itself is
replicated on every core (its PE cost is set by weight-streaming, not batch
size, so replication is free). No collectives.

Numerics: all matmuls bf16 with fp32 PSUM accumulation; LSTM cell state c is
kept fp32; h is rounded to bf16 each step (validated: end-to-end rel err
~2.5e-3 vs fp32 reference). Logits leave the device as bf16 and are cast to
fp32 on the host.

v5 structure (vs the 398us v3):
 - The x-projection (x @ W_ih^T + b_ih + b_hh, gate-permuted) is computed on
   the host in fp32 and streamed per 128-token chunk on the GpSimd (SWDGE)
   DMA queue, 6 chunks deep. The per-step PSUM inject stays as col-tiled
   identity-selector pairs (a full-array M=128 inject was tried and broke
   the col-group concurrency of the following gate pairs: +30us).
 - Vocab filler is balanced to exactly 10 matmuls per step (kc-accumulation
   of a tile may span the two steps of its chunk), so the PE no longer runs
   dry before the h transposes on the lighter steps.
 - tanh(c) is computed in two 128-col halves so h's first half (and its PE
   transpose) starts ~400ns earlier.
 - Startup: whh/wvt are split across the sync and SWDGE queues so the first
   gate matmul starts ~10us earlier.
"""
import os
import sys
import types

import numpy as np

import concourse.bass as bass
import concourse.bacc as bacc
import concourse.mybir as mybir
from concourse import tile
from concourse.bass_utils import run_bass_kernel_spmd

dt = mybir.dt
AF = mybir.ActivationFunctionType

B, T = 64, 64
Q, E, H, V = 512, 256, 512, 20000
NCORES = 8
VS = V // NCORES          # 2500 vocab rows per core
TB = T * B                # 4096 tokens
NVT = 5                   # vocab N-tiles per 128-token chunk
VT = VS // NVT            # 500
NCH = TB // 128           # 32 token chunks
XP_PREFETCH = 6           # xp chunk DMA lookahead (== xps pool depth)
START_IDX = 1


def _gate_perm():
    """new gate-column index -> original gate-column index.

    bank0 = [i|g] (both inputs of the early i*g product), bank1 = [f|o]
    (consumed late in the chain), so the c-update critical path starts as
    soon as bank1's matmuls land."""
    gate_of = {0: (0, 2), 1: (1, 3)}   # bank -> (q for j<256, q for j>=256)
    perm = np.empty(4 * H, dtype=np.int64)
    for bank in range(2):
        for hh in range(2):
            for j in range(512):
                q = gate_of[bank][1 if j >= 256 else 0]
                u = 256 * hh + (j % 256)
                perm[bank * 1024 + hh * 512 + j] = q * H + u
    return perm


def build(nc):
    f32, bf16 = dt.float32, dt.bfloat16

    xpt_d = nc.declare_dram_parameter("xpt", [TB, 4 * H], bf16, isOutput=False)
    whh_d = nc.declare_dram_parameter("whh", [H, 4 * H], bf16, isOutput=False)
    h0t_d = nc.declare_dram_parameter("h0t", [128, 4 * B], bf16, isOutput=False)
    c0_d = nc.declare_dram_parameter("c0t", [128, 256], bf16, isOutput=False)
    ident_d = nc.declare_dram_parameter("ident", [128, 128], bf16, isOutput=False)
    wvt_d = nc.declare_dram_parameter("wvt", [H + 1, VS], bf16, isOutput=False)
    out_d = nc.declare_dram_parameter("out", [TB, VS], bf16, isOutput=True)

    with tile.TileContext(nc) as tc:
        with (
            tc.tile_pool(name="const", bufs=1) as const,
            tc.tile_pool(name="work", bufs=2) as work,
            tc.tile_pool(name="hbf", bufs=2) as hpool,
            tc.tile_pool(name="xps", bufs=XP_PREFETCH) as xps,
            tc.tile_pool(name="stage", bufs=2) as stpool,
            tc.tile_pool(name="pgate", bufs=2, space="PSUM") as pg,
            tc.tile_pool(name="ptrans", bufs=1, space="PSUM") as pt,
            tc.tile_pool(name="pvocab", bufs=3, space="PSUM") as pv,
        ):
            # ---- constant uploads, split across the two DMA queues so the
            # recurrence can start as soon as whh + xp chunk 0 land ---------
            whh = const.tile([128, 4 * 4 * H], bf16)        # [128, (kc, gatecol)]
            whh_v = whh[:].rearrange("p (c n) -> p c n", c=4)
            whh_dv = whh_d[:].rearrange("(c p) n -> p c n", p=128)
            wvt = const.tile([128, 4 * VS], bf16)           # [128, (kc, vocab)]
            wvt_v = wvt[:].rearrange("p (c n) -> p c n", c=4)
            wvt_dv = wvt_d[0:H, :].rearrange("(c p) n -> p c n", p=128)

            xp_tiles = {}

            def emit_xp_dma(c, eng=None):
                xp_tiles[c] = xps.tile([128, 4 * H], bf16, tag="xp", name=f"xp{c}")
                (eng or nc.gpsimd).dma_start(
                    xp_tiles[c][:], xpt_d[128 * c : 128 * c + 128, :]
                )

            ident = const.tile([128, 128], bf16)
            nc.sync.dma_start(ident[:], ident_d[:])
            h0T = const.tile([128, 4 * B], bf16)
            nc.sync.dma_start(h0T[:], h0t_d[:])
            c_t = const.tile([128, 256], bf16)
            nc.sync.dma_start(c_t[:], c0_d[:])
            emit_xp_dma(0)                                  # gpsimd queue
            nc.sync.dma_start(whh_v[:, 0:1, :], whh_dv[:, 0:1, :])
            nc.gpsimd.dma_start(whh_v[:, 2:3, :], whh_dv[:, 2:3, :])
            nc.sync.dma_start(whh_v[:, 1:2, :], whh_dv[:, 1:2, :])
            nc.gpsimd.dma_start(whh_v[:, 3:4, :], whh_dv[:, 3:4, :])
            emit_xp_dma(1, nc.sync)
            nc.gpsimd.dma_start(wvt_v[:, 2:3, :], wvt_dv[:, 2:3, :])
            nc.sync.dma_start(wvt_v[:, 0:1, :], wvt_dv[:, 0:1, :])
            nc.gpsimd.dma_start(wvt_v[:, 3:4, :], wvt_dv[:, 3:4, :])
            nc.sync.dma_start(wvt_v[:, 1:2, :], wvt_dv[:, 1:2, :])
            for c0_ in range(2, XP_PREFETCH):
                emit_xp_dma(c0_, nc.sync if c0_ % 2 else None)

            H_allT = const.tile([128, 4 * TB], bf16)        # [128, (kc, token)]

            # ---- PE warmup: ~5us of dummy matmuls so the HAM clock gate
            # un-throttles (1.2 -> 2.4 GHz) while the parameter DMAs stream,
            # instead of 3.4us into the real recurrence. Results unread.
            wu = pt.tile([128, 256], f32, tag="pst", name="wu")
            for w in range(40):
                r = (w % 2) * 128
                nc.tensor.matmul(
                    wu[:, r : r + 128],
                    lhsT=ident[:],
                    rhs=ident[:],
                    start=True,
                    stop=True,
                )

            # ---- vocab projection: a flat queue of (tile, kc) matmuls,
            # drained at exactly 10 per step --------------------------------
            vocab_psum = {}
            st_tiles = {}
            mm_queue = []            # pending (m, vl, kc) in emission order

            def queue_vocab_chunk(m):
                for vl in range(NVT):
                    for kc in range(4):
                        mm_queue.append((m, vl, kc))

            def emit_vocab_mms(n):
                for _ in range(min(n, len(mm_queue))):
                    m, vl, kc = mm_queue.pop(0)
                    if kc == 0:
                        vocab_psum[(m, vl)] = pv.tile(
                            [128, VT], f32, tag="psv", name=f"psv{m}_{vl}"
                        )
                    nc.tensor.matmul(
                        vocab_psum[(m, vl)][:],
                        lhsT=H_allT[:, kc * TB + 128 * m : kc * TB + 128 * m + 128],
                        rhs=wvt[:, kc * VS + vl * VT : kc * VS + vl * VT + VT],
                        start=(kc == 0),
                        stop=(kc == 3),
                    )

            def emit_vocab_stage(m, vls):
                # plain PSUM->SBUF cast (b_vocab is folded in on the host);
                # 2 tiles per chunk on DVE, 3 on ACT so neither saturates
                if m not in st_tiles:
                    st_tiles[m] = stpool.tile(
                        [128, VS], bf16, tag="st", name=f"st{m}"
                    )
                st = st_tiles[m]
                for vl in vls:
                    dst = st[:, vl * VT : (vl + 1) * VT]
                    src = vocab_psum.pop((m, vl))[:]
                    if vl in (0, 2, 4):
                        nc.vector.tensor_copy(dst, src)
                    else:
                        nc.scalar.copy(dst, src)

            def emit_vocab_out(m):
                nc.sync.dma_start(out_d[128 * m : 128 * m + 128, :], st_tiles.pop(m)[:])

            # ---- the 64 LSTM steps ---------------------------------------------
            # Each step's two PSUM banks are opened by the xp inject (selector
            # columns of the identity pick the step's 64 token rows out of the
            # 128-token xp chunk). The inject has no h dependency, so step
            # t+1's injects are emitted BEFORE step t's h transposes: they
            # fill the PE during the ~0.4us the transposes wait on h_bf.
            def emit_inject(t):
                s = t % 2
                xpc = xp_tiles[t // 2]
                p0 = pg.tile([128, 512], f32, tag="psg")
                p1 = pg.tile([128, 512], f32, tag="psg")
                for bank, psg in ((0, p0), (1, p1)):
                    for hh in range(2):
                        n0 = bank * 1024 + hh * 512
                        nc.tensor.matmul(
                            psg[64 * hh : 64 * hh + 64, :],
                            lhsT=ident[:, s * 64 : s * 64 + 64],
                            rhs=xpc[:, n0 : n0 + 512],
                            start=True,
                            stop=False,
                            tile_position=(0, 64 * hh),
                            skip_group_check=True,
                        )
                return p0, p1

            psg_next = emit_inject(0)
            for t in range(T):
                psg0, psg1 = psg_next
                cch, s = t // 2, t % 2
                if s == 0 and cch > 0:
                    xp_tiles.pop(cch - 1)

                def lhs_h(kc, t=t):
                    if t == 0:
                        return h0T[:, kc * 64 : (kc + 1) * 64]
                    c0 = kc * TB + 64 * (t - 1)
                    return H_allT[:, c0 : c0 + 64]

                # interleave the two column-tile chains (hh=0 on cols 0-63,
                # hh=1 on cols 64-127): adjacent matmuls hit different col
                # groups and run concurrently in the PE array. N=512 per MM
                # is the sweet spot: the pair's two M=64 weight loads
                # (2 x 53ns) exactly hide under the 213ns stream; smaller N
                # goes LDWEIGHTS-bound (measured: N=256 regions cost +120us).
                for bank, psg in ((0, psg0), (1, psg1)):
                    for kc in (0, 2, 1, 3):
                        for hh in range(2):
                            n0 = bank * 1024 + hh * 512
                            nc.tensor.matmul(
                                psg[64 * hh : 64 * hh + 64, :],
                                lhsT=lhs_h(kc),
                                rhs=whh[:, kc * 2048 + n0 : kc * 2048 + n0 + 512],
                                start=False,
                                stop=(kc == 3),
                                tile_position=(0, 64 * hh),
                                skip_group_check=True,
                            )

                # activations: bank0 = [i|g] (lands first), bank1 = [f|o].
                # The whole elementwise chain runs in bf16 (2x DVE modes;
                # validated 3.9e-3 end-to-end rel err vs 2.1e-3 with f32 c).
                s_ig = work.tile([128, 512], bf16, tag="s_ig")
                s_fo = work.tile([128, 512], bf16, tag="s_fo")
                igt = work.tile([128, 256], bf16, tag="igt")
                fct = work.tile([128, 256], bf16, tag="fct")
                tct = work.tile([128, 256], bf16, tag="tct")
                h_bf = hpool.tile([128, 256], bf16, tag="h")

                nc.scalar.activation(s_ig[:, 0:256], psg0[:, 0:256], AF.Sigmoid)
                nc.scalar.activation(s_ig[:, 256:512], psg0[:, 256:512], AF.Tanh)
                nc.vector.tensor_mul(igt[:], s_ig[:, 0:256], s_ig[:, 256:512])
                # sigmoid(f) in 128-col halves so fct0 starts one ACT-op
                # earlier; c-update halves pipeline with the tanh halves below
                nc.scalar.activation(s_fo[:, 0:128], psg1[:, 0:128], AF.Sigmoid)
                nc.scalar.activation(s_fo[:, 128:256], psg1[:, 128:256], AF.Sigmoid)
                nc.scalar.activation(s_fo[:, 256:512], psg1[:, 256:512], AF.Sigmoid)
                for ui in range(2):
                    cs = slice(ui * 128, (ui + 1) * 128)
                    nc.vector.tensor_mul(fct[:, cs], s_fo[:, cs], c_t[:, cs])
                    nc.vector.tensor_add(c_t[:, cs], fct[:, cs], igt[:, cs])

                # lagged vocab matmuls fill the PE while this step's
                # activation chain runs; exactly 10 MMs per step
                if t >= 2 and t % 2 == 0:
                    queue_vocab_chunk(t // 2 - 1)
                emit_vocab_mms(11 if t % 2 == 0 else 9)
                if t + 1 < T:
                    psg_next = emit_inject(t + 1)

                # tanh(c) in two halves; h's first half (and its transpose)
                # starts while the second tanh half is still on ACT
                pst = pt.tile([128, 256], f32, tag="pst")
                H_v = H_allT[:].rearrange("p (c n) -> p c n", c=4)
                for ui in range(2):
                    nc.scalar.activation(
                        tct[:, ui * 128 : (ui + 1) * 128],
                        c_t[:, ui * 128 : (ui + 1) * 128],
                        AF.Tanh,
                    )
                    nc.vector.tensor_mul(
                        h_bf[:, ui * 128 : (ui + 1) * 128],
                        s_fo[:, 256 + ui * 128 : 256 + (ui + 1) * 128],
                        tct[:, ui * 128 : (ui + 1) * 128],
                    )
                    nc.tensor.matmul(
                        pst[:, ui * 128 : (ui + 1) * 128],
                        lhsT=h_bf[:, ui * 128 : (ui + 1) * 128],
                        rhs=ident[:],
                        start=True,
                        stop=True,
                    )
                    if ui == 0:
                        nc.vector.tensor_copy(
                            H_v[:, 0:3:2, 64 * t : 64 * t + 64],
                            pst[:, 0:128].rearrange("p (c n) -> p c n", c=2),
                        )
                    else:
                        nc.scalar.copy(
                            H_v[:, 1:4:2, 64 * t : 64 * t + 64],
                            pst[:, 128:256].rearrange("p (c n) -> p c n", c=2),
                        )

                # next xp chunk DMA (SWDGE queue): chunk cch+PREFETCH reuses
                # chunk cch's buffer, so emit only after this chunk's last
                # inject (the s==1 one, earlier this iteration) is in the IR
                if s == 1 and cch + XP_PREFETCH < NCH:
                    emit_xp_dma(cch + XP_PREFETCH)
                # staging casts sit behind the chain ops in each engine FIFO
                if t >= 2:
                    m = t // 2 - 1
                    if t % 2 == 0:
                        emit_vocab_stage(m, (0, 1))
                    else:
                        emit_vocab_stage(m, (2, 3, 4))
                        emit_vocab_out(m)

            # tail: last vocab chunk, stage->DMA pipelined per tile
            m = NCH - 1
            queue_vocab_chunk(m)
            emit_vocab_mms(20)
            for vl in range(NVT):
                # scalar HWDGE queue: the sync queue is still draining chunk
                # 30's full-row out DMA at this point
                emit_vocab_stage(m, (vl,))
                nc.scalar.dma_start(
                    out_d[128 * m : 128 * m + 128, vl * VT : (vl + 1) * VT],
                    st_tiles[m][:, vl * VT : (vl + 1) * VT],
                )
            st_tiles.pop(m)


def _host_prep(inputs):
    import ml_dtypes

    bf = ml_dtypes.bfloat16
    f32 = np.float32

    qv = inputs["question_vectors"].astype(f32)
    emb = inputs["emb_table"].astype(f32)
    W_h, W_c = inputs["W_h"].astype(f32), inputs["W_c"].astype(f32)
    W_ih, W_hh = inputs["W_ih"].astype(f32), inputs["W_hh"].astype(f32)
    b_ih, b_hh = inputs["b_ih"].astype(f32), inputs["b_hh"].astype(f32)
    W_vocab, b_vocab = inputs["W_vocab"].astype(f32), inputs["b_vocab"].astype(f32)
    answers = inputs["answers"]

    perm = _gate_perm()
    whh = np.ascontiguousarray(W_hh.T[:, perm]).astype(bf)      # [512, 2048]

    # teacher-forced inputs gathered on host, then the x-projection
    # (x @ W_ih^T + b_ih + b_hh) in fp32, gate-permuted: [TB, 2048]
    xs = np.concatenate(
        [
            np.broadcast_to(emb[START_IDX], (1, B, E)),
            emb[answers[:, :-1]].transpose(1, 0, 2),
        ],
        axis=0,
    ).reshape(TB, E)
    xpt = np.ascontiguousarray(
        (xs @ W_ih.T + (b_ih + b_hh))[:, perm]
    ).astype(bf)                                                # [TB, 2048]

    # initial state projections, pre-tiled for the device layouts
    h0 = qv @ W_h.T                                             # [B, H]
    c0 = qv @ W_c.T                                             # [B, H]
    h0t = np.ascontiguousarray(
        h0.T.reshape(4, 128, B).transpose(1, 0, 2).reshape(128, 4 * B)
    ).astype(bf)                                                # [128,(kc,b)]
    c0t = np.ascontiguousarray(
        c0.reshape(B, 2, 256).transpose(1, 0, 2).reshape(128, 256)
    ).astype(bf)                                                # [(hh,b),256]

    ident = np.eye(128, dtype=bf)

    common = dict(xpt=xpt, whh=whh, h0t=h0t, c0t=c0t, ident=ident)
    in_maps = []
    for k in range(NCORES):
        wvt = np.concatenate(
            [W_vocab[k * VS : (k + 1) * VS].T, b_vocab[None, k * VS : (k + 1) * VS]],
            axis=0,
        ).astype(bf)                                        # [513, 2500]
        in_maps.append(dict(common, wvt=wvt))
    return in_maps


def _install_ntff_hook():
    """Shim antenv.axon_hooks (absent in this image) so BASS_TRACE=1 works."""
    if "antenv.axon_hooks" in sys.modules:
        return
    try:
        mod = types.ModuleType("antenv.axon_hooks")
        mod._hook = None
        mod.set_axon_ntff_profile_hook = lambda h: setattr(mod, "_hook", h)
        mod.get_axon_ntff_profile_hook = lambda: mod._hook
        sys.modules["antenv.axon_hooks"] = mod
        from trn_agent_boot.trn_boot import _ntff_profile_via_ctypes

        mod.set_axon_ntff_profile_hook(
            _ntff_profile_via_ctypes("/opt/axon/libaxon_pjrt.so")
        )
    except Exception:
        sys.modules.pop("antenv.axon_hooks", None)


def kernel(**inputs):
    inputs = {k: np.asarray(v) for k, v in inputs.items()}
    if os.environ.get("BASS_TRACE"):
        _install_ntff_hook()

    in_maps = _host_prep(inputs)

    nc = bacc.Bacc("TRN2", target_bir_lowering=False, debug=False, num_devices=NCORES)
    build(nc)
    nc.compile()

    res = run_bass_kernel_spmd(nc, in_maps, core_ids=list(range(NCORES)))
    kernel._last_result = res

    b_vocab = inputs["b_vocab"].astype(np.float32)
    outs = [
        res.results[k]["out"].astype(np.float32).reshape(T, B, VS).transpose(1, 0, 2)
        + b_vocab[k * VS : (k + 1) * VS]
        for k in range(NCORES)
    ]
    return np.concatenate(outs, axis=2)


# revision 40
# speedup vs baseline: 1.0405x; 1.0336x over previous
"""Trainium2 Bass kernel for nn_AnswerDecoder (LSTM decoder + vocab projection).

Sharding: vocab-parallel across 8 NeuronCores (each core owns V/8 = 2500 rows
of W_vocab and produces logits[:, :, k*2500:(k+1)*2500]); the LSTM itself is
replicated on every core (its PE cost is set by weight-streaming, not batch
size, so replication is free). No collectives.

Numerics: all matmuls bf16 with fp32 PSUM accumulation; LSTM cell state c is
kept fp32; h is rounded to bf16 each step (validated: end-to-end rel err
~2.5e-3 vs fp32 reference). Logits leave the device as bf16 and are cast to
fp32 on the host.

v5 structure (vs the 398us v3):
 - The x-projection (x @ W_ih^T + b_ih + b_hh, gate-permuted) is computed on
   the host in fp32 and streamed per 128-token chunk on the GpSimd (SWDGE)
   DMA queue, 6 chunks deep. The per-step PSUM inject stays as col-tiled
   identity-selector pairs (a full-array M=128 inject was tried and broke
   the col-group concurrency of the following gate pairs: +30us).
 - Vocab filler is balanced to exactly 10 matmuls per step (kc-accumulation
   of a tile may span the two steps of its chunk), so the PE no longer runs
   dry before the h transposes on the lighter steps.
 - tanh(c) is computed in two 128-col halves so h's first half (and its PE
   transpose) starts ~400ns earlier.
 - Startup: whh/wvt are split across the sync and SWDGE queues so the first
   gate matmul starts ~10us earlier.
"""
import os
import sys
import types

import numpy as np

import concourse.bass as bass
import concourse.bacc as bacc
import concourse.mybir as mybir
from concourse import tile
from concourse.bass_utils import run_bass_kernel_spmd

dt = mybir.dt
AF = mybir.ActivationFunctionType

B, T = 64, 64
Q, E, H, V = 512, 256, 512, 20000
NCORES = 8
VS = V // NCORES          # 2500 vocab rows per core
TB = T * B                # 4096 tokens
NVT = 5                   # vocab N-tiles per 128-token chunk
VT = VS // NVT            # 500
NCH = TB // 128           # 32 token chunks
XP_PREFETCH = 6           # xp chunk DMA lookahead (== xps pool depth)
START_IDX = 1


def _gate_perm():
    """new gate-column index -> original gate-column index.

    bank0 = [i|g] (both inputs of the early i*g product), bank1 = [f|o]
    (consumed late in the chain), so the c-update critical path starts as
    soon as bank1's matmuls land."""
    gate_of = {0: (0, 2), 1: (1, 3)}   # bank -> (q for j<256, q for j>=256)
    perm = np.empty(4 * H, dtype=np.int64)
    for bank in range(2):
        for hh in range(2):
            for j in range(512):
                q = gate_of[bank][1 if j >= 256 else 0]
                u = 256 * hh + (j % 256)
                perm[bank * 1024 + hh * 512 + j] = q * H + u
    return perm


def build(nc):
    f32, bf16 = dt.float32, dt.bfloat16

    xpt_d = nc.declare_dram_parameter("xpt", [TB, 4 * H], bf16, isOutput=False)
    whh_d = nc.declare_dram_parameter("whh", [H, 4 * H], bf16, isOutput=False)
    h0t_d = nc.declare_dram_parameter("h0t", [128, 4 * B], bf16, isOutput=False)
    c0_d = nc.declare_dram_parameter("c0t", [128, 256], bf16, isOutput=False)
    ident_d = nc.declare_dram_parameter("ident", [128, 128], bf16, isOutput=False)
    wvt_d = nc.declare_dram_parameter("wvt", [H + 1, VS], bf16, isOutput=False)
    out_d = nc.declare_dram_parameter("out", [TB, VS], bf16, isOutput=True)

    with tile.TileContext(nc) as tc:
        with (
            tc.tile_pool(name="const", bufs=1) as const,
            tc.tile_pool(name="work", bufs=2) as work,
            tc.tile_pool(name="hbf", bufs=2) as hpool,
            tc.tile_pool(name="xps", bufs=XP_PREFETCH) as xps,
            tc.tile_pool(name="stage", bufs=2) as stpool,
            tc.tile_pool(name="pgate", bufs=2, space="PSUM") as pg,
            tc.tile_pool(name="ptrans", bufs=1, space="PSUM") as pt,
            tc.tile_pool(name="pvocab", bufs=3, space="PSUM") as pv,
        ):
            # ---- constant uploads, split across the two DMA queues so the
            # recurrence can start as soon as whh + xp chunk 0 land ---------
            whh = const.tile([128, 4 * 4 * H], bf16)        # [128, (kc, gatecol)]
            whh_v = whh[:].rearrange("p (c n) -> p c n", c=4)
            whh_dv = whh_d[:].rearrange("(c p) n -> p c n", p=128)
            wvt = const.tile([128, 4 * VS], bf16)           # [128, (kc, vocab)]
            wvt_v = wvt[:].rearrange("p (c n) -> p c n", c=4)
            wvt_dv = wvt_d[0:H, :].rearrange("(c p) n -> p c n", p=128)

            xp_tiles = {}

            def emit_xp_dma(c, eng=None):
                xp_tiles[c] = xps.tile([128, 4 * H], bf16, tag="xp", name=f"xp{c}")
                (eng or nc.gpsimd).dma_start(
                    xp_tiles[c][:], xpt_d[128 * c : 128 * c + 128, :]
                )

            ident = const.tile([128, 128], bf16)
            nc.sync.dma_start(ident[:], ident_d[:])
            h0T = const.tile([128, 4 * B], bf16)
            nc.sync.dma_start(h0T[:], h0t_d[:])
            c_t = const.tile([128, 256], bf16)
            nc.sync.dma_start(c_t[:], c0_d[:])
            emit_xp_dma(0)                                  # gpsimd queue
            nc.sync.dma_start(whh_v[:, 0:1, :], whh_dv[:, 0:1, :])
            nc.gpsimd.dma_start(whh_v[:, 2:3, :], whh_dv[:, 2:3, :])
            nc.sync.dma_start(whh_v[:, 1:2, :], whh_dv[:, 1:2, :])
            nc.gpsimd.dma_start(whh_v[:, 3:4, :], whh_dv[:, 3:4, :])
            emit_xp_dma(1, nc.sync)
            nc.gpsimd.dma_start(wvt_v[:, 2:3, :], wvt_dv[:, 2:3, :])
            nc.sync.dma_start(wvt_v[:, 0:1, :], wvt_dv[:, 0:1, :])
            nc.gpsimd.dma_start(wvt_v[:, 3:4, :], wvt_dv[:, 3:4, :])
            nc.sync.dma_start(wvt_v[:, 1:2, :], wvt_dv[:, 1:2, :])
            for c0_ in range(2, XP_PREFETCH):
                emit_xp_dma(c0_, nc.sync if c0_ % 2 else None)

            H_allT = const.tile([128, 4 * TB], bf16)        # [128, (kc, token)]

            # ---- PE warmup: ~5us of dummy matmuls so the HAM clock gate
            # un-throttles (1.2 -> 2.4 GHz) while the parameter DMAs stream,
            # instead of 3.4us into the real recurrence. Results unread.
            wu = pt.tile([128, 256], f32, tag="pst", name="wu")
            for w in range(40):
                r = (w % 2) * 128
                nc.tensor.matmul(
                    wu[:, r : r + 128],
                    lhsT=ident[:],
                    rhs=ident[:],
                    start=True,
                    stop=True,
                )

            # ---- vocab projection: a flat queue of (tile, kc) matmuls,
            # drained at exactly 10 per step --------------------------------
            vocab_psum = {}
            st_tiles = {}
            mm_queue = []            # pending (m, vl, kc) in emission order

            def queue_vocab_chunk(m):
                for vl in range(NVT):
                    for kc in range(4):
                        mm_queue.append((m, vl, kc))

            def emit_vocab_mms(n):
                for _ in range(min(n, len(mm_queue))):
                    m, vl, kc = mm_queue.pop(0)
                    if kc == 0:
                        vocab_psum[(m, vl)] = pv.tile(
                            [128, VT], f32, tag="psv", name=f"psv{m}_{vl}"
                        )
                    nc.tensor.matmul(
                        vocab_psum[(m, vl)][:],
                        lhsT=H_allT[:, kc * TB + 128 * m : kc * TB + 128 * m + 128],
                        rhs=wvt[:, kc * VS + vl * VT : kc * VS + vl * VT + VT],
                        start=(kc == 0),
                        stop=(kc == 3),
                    )

            def emit_vocab_stage(m, vls):
                # plain PSUM->SBUF cast (b_vocab is folded in on the host);
                # 2 tiles per chunk on DVE, 3 on ACT so neither saturates
                if m not in st_tiles:
                    st_tiles[m] = stpool.tile(
                        [128, VS], bf16, tag="st", name=f"st{m}"
                    )
                st = st_tiles[m]
                for vl in vls:
                    dst = st[:, vl * VT : (vl + 1) * VT]
                    src = vocab_psum.pop((m, vl))[:]
                    if vl in (0, 2, 4):
                        nc.vector.tensor_copy(dst, src)
                    else:
                        nc.scalar.copy(dst, src)

            def emit_vocab_out(m):
                nc.sync.dma_start(out_d[128 * m : 128 * m + 128, :], st_tiles.pop(m)[:])

            # ---- the 64 LSTM steps ---------------------------------------------
            # Each step's two PSUM banks are opened by the xp inject (selector
            # columns of the identity pick the step's 64 token rows out of the
            # 128-token xp chunk). The inject has no h dependency, so step
            # t+1's injects are emitted BEFORE step t's h transposes: they
            # fill the PE during the ~0.4us the transposes wait on h_bf.
            def emit_inject(t):
                s = t % 2
                xpc = xp_tiles[t // 2]
                p0 = pg.tile([128, 512], f32, tag="psg")
                p1 = pg.tile([128, 512], f32, tag="psg")
                for bank, psg in ((0, p0), (1, p1)):
                    for hh in range(2):
                        n0 = bank * 1024 + hh * 512
                        nc.tensor.matmul(
                            psg[64 * hh : 64 * hh + 64, :],
                            lhsT=ident[:, s * 64 : s * 64 + 64],
                            rhs=xpc[:, n0 : n0 + 512],
                            start=True,
                            stop=False,
                            tile_position=(0, 64 * hh),
                            skip_group_check=True,
                        )
                return p0, p1

            psg_next = emit_inject(0)
            for t in range(T):
                psg0, psg1 = psg_next
                cch, s = t // 2, t % 2
                if s == 0 and cch > 0:
                    xp_tiles.pop(cch - 1)

                def lhs_h(kc, t=t):
                    if t == 0:
                        return h0T[:, kc * 64 : (kc + 1) * 64]
                    c0 = kc * TB + 64 * (t - 1)
                    return H_allT[:, c0 : c0 + 64]

                # interleave the two column-tile chains (hh=0 on cols 0-63,
                # hh=1 on cols 64-127): adjacent matmuls hit different col
                # groups and run concurrently in the PE array. N=512 per MM
                # is the sweet spot: the pair's two M=64 weight loads
                # (2 x 53ns) exactly hide under the 213ns stream; smaller N
                # goes LDWEIGHTS-bound (measured: N=256 regions cost +120us).
                for bank, psg in ((0, psg0), (1, psg1)):
                    for kc in (0, 2, 1, 3):
                        for hh in range(2):
                            n0 = bank * 1024 + hh * 512
                            nc.tensor.matmul(
                                psg[64 * hh : 64 * hh + 64, :],
                                lhsT=lhs_h(kc),
                                rhs=whh[:, kc * 2048 + n0 : kc * 2048 + n0 + 512],
                                start=False,
                                stop=(kc == 3),
                                tile_position=(0, 64 * hh),
                                skip_group_check=True,
                            )

                # activations: bank0 = [i|g] (lands first), bank1 = [f|o].
                # The whole elementwise chain runs in bf16 (2x DVE modes;
                # validated 3.9e-3 end-to-end rel err vs 2.1e-3 with f32 c).
                s_ig = work.tile([128, 512], bf16, tag="s_ig")
                s_fo = work.tile([128, 512], bf16, tag="s_fo")
                igt = work.tile([128, 256], bf16, tag="igt")
                fct = work.tile([128, 256], bf16, tag="fct")
                tct = work.tile([128, 256], bf16, tag="tct")
                h_bf = hpool.tile([128, 256], bf16, tag="h")

                nc.scalar.activation(s_ig[:, 0:256], psg0[:, 0:256], AF.Sigmoid)
                nc.scalar.activation(s_ig[:, 256:512], psg0[:, 256:512], AF.Tanh)
                nc.vector.tensor_mul(igt[:], s_ig[:, 0:256], s_ig[:, 256:512])
                # sigmoid(f) in 128-col halves so fct0 starts one ACT-op
                # earlier; c-update halves pipeline with the tanh halves below
                nc.scalar.activation(s_fo[:, 0:128], psg1[:, 0:128], AF.Sigmoid)
                nc.scalar.activation(s_fo[:, 128:256], psg1[:, 128:256], AF.Sigmoid)
                nc.scalar.activation(s_fo[:, 256:512], psg1[:, 256:512], AF.Sigmoid)
                for ui in range(2):
                    cs = slice(ui * 128, (ui + 1) * 128)
                    nc.vector.tensor_mul(fct[:, cs], s_fo[:, cs], c_t[:, cs])
                    nc.vector.tensor_add(c_t[:, cs], fct[:, cs], igt[:, cs])

                # lagged vocab matmuls fill the PE while this step's
                # activation chain runs; exactly 10 MMs per step
                if t >= 2 and t % 2 == 0:
                    queue_vocab_chunk(t // 2 - 1)
                emit_vocab_mms(9 if t % 2 == 0 else 11)
                if t + 1 < T:
                    psg_next = emit_inject(t + 1)

                # tanh(c) in two halves; h's first half (and its transpose)
                # starts while the second tanh half is still on ACT
                pst = pt.tile([128, 256], f32, tag="pst")
                H_v = H_allT[:].rearrange("p (c n) -> p c n", c=4)
                for ui in range(2):
                    nc.scalar.activation(
                        tct[:, ui * 128 : (ui + 1) * 128],
                        c_t[:, ui * 128 : (ui + 1) * 128],
                        AF.Tanh,
                    )
                    nc.vector.tensor_mul(
                        h_bf[:, ui * 128 : (ui + 1) * 128],
                        s_fo[:, 256 + ui * 128 : 256 + (ui + 1) * 128],
                        tct[:, ui * 128 : (ui + 1) * 128],
                    )
                    nc.tensor.matmul(
                        pst[:, ui * 128 : (ui + 1) * 128],
                        lhsT=h_bf[:, ui * 128 : (ui + 1) * 128],
                        rhs=ident[:],
                        start=True,
                        stop=True,
                    )
                    if ui == 0:
                        nc.vector.tensor_copy(
                            H_v[:, 0:3:2, 64 * t : 64 * t + 64],
                            pst[:, 0:128].rearrange("p (c n) -> p c n", c=2),
                        )
                    else:
                        nc.scalar.copy(
                            H_v[:, 1:4:2, 64 * t : 64 * t + 64],
                            pst[:, 128:256].rearrange("p (c n) -> p c n", c=2),
                        )

                # next xp chunk DMA (SWDGE queue): chunk cch+PREFETCH reuses
                # chunk cch's buffer, so emit only after this chunk's last
                # inject (the s==1 one, earlier this iteration) is in the IR
                if s == 1 and cch + XP_PREFETCH < NCH:
                    emit_xp_dma(cch + XP_PREFETCH)
                # staging casts sit behind the chain ops in each engine FIFO
                if t >= 2:
                    m = t // 2 - 1
                    if t % 2 == 0:
                        emit_vocab_stage(m, (0, 1))
                    else:
                        emit_vocab_stage(m, (2, 3, 4))
                        emit_vocab_out(m)

            # tail: last vocab chunk, stage->DMA pipelined per tile
            m = NCH - 1
            queue_vocab_chunk(m)
            emit_vocab_mms(20)
            for vl in range(NVT):
                # scalar HWDGE queue: the sync queue is still draining chunk
                # 30's full-row out DMA at this point
                emit_vocab_stage(m, (vl,))
                nc.scalar.dma_start(
                    out_d[128 * m : 128 * m + 128, vl * VT : (vl + 1) * VT],
                    st_tiles[m][:, vl * VT : (vl + 1) * VT],
                )
            st_tiles.pop(m)


def _host_prep(inputs):
    import ml_dtypes

    bf = ml_dtypes.bfloat16
    f32 = np.float32

    qv = inputs["question_vectors"].astype(f32)
    emb = inputs["emb_table"].astype(f32)
    W_h, W_c = inputs["W_h"].astype(f32), inputs["W_c"].astype(f32)
    W_ih, W_hh = inputs["W_ih"].astype(f32), inputs["W_hh"].astype(f32)
    b_ih, b_hh = inputs["b_ih"].astype(f32), inputs["b_hh"].astype(f32)
    W_vocab, b_vocab = inputs["W_vocab"].astype(f32), inputs["b_vocab"].astype(f32)
    answers = inputs["answers"]

    perm = _gate_perm()
    whh = np.ascontiguousarray(W_hh.T[:, perm]).astype(bf)      # [512, 2048]

    # teacher-forced inputs gathered on host, then the x-projection
    # (x @ W_ih^T + b_ih + b_hh) in fp32, gate-permuted: [TB, 2048]
    xs = np.concatenate(
        [
            np.broadcast_to(emb[START_IDX], (1, B, E)),
            emb[answers[:, :-1]].transpose(1, 0, 2),
        ],
        axis=0,
    ).reshape(TB, E)
    xpt = np.ascontiguousarray(
        (xs @ W_ih.T + (b_ih + b_hh))[:, perm]
    ).astype(bf)                                                # [TB, 2048]

    # initial state projections, pre-tiled for the device layouts
    h0 = qv @ W_h.T                                             # [B, H]
    c0 = qv @ W_c.T                                             # [B, H]
    h0t = np.ascontiguousarray(
        h0.T.reshape(4, 128, B).transpose(1, 0, 2).reshape(128, 4 * B)
    ).astype(bf)                                                # [128,(kc,b)]
    c0t = np.ascontiguousarray(
        c0.reshape(B, 2, 256).transpose(1, 0, 2).reshape(128, 256)
    ).astype(bf)                                                # [(hh,b),256]

    ident = np.eye(128, dtype=bf)

    common = dict(xpt=xpt, whh=whh, h0t=h0t, c0t=c0t, ident=ident)
    in_maps = []
    for k in range(NCORES):
        wvt = np.concatenate(
            [W_vocab[k * VS : (k + 1) * VS].T, b_vocab[None, k * VS : (k + 1) * VS]],
            axis=0,
        ).astype(bf)                                        # [513, 2500]
        in_maps.append(dict(common, wvt=wvt))
    return in_maps


def _install_ntff_hook():
    """Shim antenv.axon_hooks (absent in this image) so BASS_TRACE=1 works."""
    if "antenv.axon_hooks" in sys.modules:
        return
    try:
        mod = types.ModuleType("antenv.axon_hooks")
        mod._hook = None
        mod.set_axon_ntff_profile_hook = lambda h: setattr(mod, "_hook", h)
        mod.get_axon_ntff_profile_hook = lambda: mod._hook
        sys.modules["antenv.axon_hooks"] = mod
        from trn_agent_boot.trn_boot import _ntff_profile_via_ctypes

        mod.set_axon_ntff_profile_hook(
            _ntff_profile_via_ctypes("/opt/axon/libaxon_pjrt.so")
        )
    except Exception:
        sys.modules.pop("antenv.axon_hooks", None)


def kernel(**inputs):
    inputs = {k: np.asarray(v) for k, v in inputs.items()}
    if os.environ.get("BASS_TRACE"):
        _install_ntff_hook()

    in_maps = _host_prep(inputs)

    nc = bacc.Bacc("TRN2", target_bir_lowering=False, debug=False, num_devices=NCORES)
    build(nc)
    nc.compile()

    res = run_bass_kernel_spmd(nc, in_maps, core_ids=list(range(NCORES)))
    kernel._last_result = res

    b_vocab = inputs["b_vocab"].astype(np.float32)
    outs = [
        res.results[k]["out"].astype(np.float32).reshape(T, B, VS).transpose(1, 0, 2)
        + b_vocab[k * VS : (k + 1) * VS]
        for k in range(NCORES)
    ]
    return np.concatenate(outs, axis=2)
